# revision 9
# baseline (speedup 1.0000x reference)
"""AVWGCN Bass kernel for 8 TRN2 NeuronCores.

Strategy: shard the node dimension N=2048 into 8 slices of 256. Host-side we
np.roll E (node rows) and X (node axis) per core so every core runs an
IDENTICAL program that computes output for its first 256 (rolled) nodes.

Math (per core, R = rolled nodes [0:256)):
  G = E@E.T (symmetric), M = exp(relu(G)) (symmetric), D = rowsum(M),
  A = M/D (row-softmax of relu(G) -- matches jax.nn.softmax up to max-shift).
  AT_col[m, r]   = A[R[r], m]           (PE transpose of A's first 2 row tiles)
  PT2[m, r]      = sum_s A[s,m]*AT_col[s,r] = (A^2)[R[r], m]   (no transposes!)
  Per batch b:   hop1[i, r] = sum_m X[b,m,i]*AT_col[m,r] = (A @ X_b)[R[r], i]
                 hop2[i, r] = sum_m X[b,m,i]*2*PT2[m,r] = 2(A^2 @ X_b)[R[r], i]
                 xt[i, r]   = X[b, R[r], i]             (PE transpose)
  H layout: H01[(k i)=128, b, r] = [xt | hop1], H2[(i)=64, b, r] = hop2
  Weights: Wp_eff[d,0] = Wp[d,0]-Wp[d,2] folds the "-I" of T2 = 2A^2 - I.
  Z: out_wide[r, (d o)] = sum_{ki} H[(ki),(b r)] * Wp_wide[(ki),(d o)]
  out[b, n=R[r], o] = bias[n,o] + sum_d E[n,d] * out_wide[r, (d,o)]
"""

import os
import sys
import time

sys.path.insert(0, "/opt/trn_rl_repo")

import numpy as np

N_CORES = 8
B, N, CIN, COUT, K, D = 64, 2048, 64, 64, 3, 16
NL = N // N_CORES  # 256 nodes per core
P = 128

_CACHE = {}
LAST_RESULT = None


def _build_bass():
    import concourse.bass as bass
    import concourse.mybir as mybir
    import concourse.tile as tile
    from concourse import bacc
    from concourse.masks import make_identity

    f32 = mybir.dt.float32
    f32r = mybir.dt.float32r
    Alu = mybir.AluOpType
    AFT = mybir.ActivationFunctionType
    AX = mybir.AxisListType

    def r_(ap):
        return ap.bitcast(f32r)

    nc = bacc.Bacc(
        "TRN2",
        target_bir_lowering=False,
        debug=False,
        enable_asserts=False,
        num_devices=N_CORES,
    )

    x_ap = nc.dram_tensor("X", [B, N, CIN], f32, kind="ExternalInput").ap()
    e_ap = nc.dram_tensor("E", [N, D], f32, kind="ExternalInput").ap()
    wp_ap = nc.dram_tensor("WP", [D, K, CIN, COUT], f32, kind="ExternalInput").ap()
    bp_ap = nc.dram_tensor("BP", [D, COUT], f32, kind="ExternalInput").ap()
    out_ap = nc.dram_tensor("OUT", [B, NL, COUT], f32, kind="ExternalOutput").ap()

    NT = N // P  # 16 row tiles
    DO = D * COUT  # 1024
    RT = NL // P  # 2 r-half tiles

    bf16 = mybir.dt.bfloat16

    with tile.TileContext(nc) as tc:
        with tc.tile_pool(name="persist", bufs=1) as pp:
            # ---- phase 0: constants / small tensors ----
            with tc.tile_pool(name="ppsum", bufs=2, space="PSUM") as pps:
                ident_f = pp.tile([P, P], f32, tag="ident_f")
                make_identity(nc, ident_f[:])
                ident = pp.tile([P, P], f32r, tag="ident")
                nc.vector.tensor_copy(ident[:], ident_f[:])

                et = pp.tile([D, N], f32r, tag="et")  # E.T [16, 2048]
                for t in range(NT):
                    etmp = pp.tile([P, D], f32r, tag="etmp")
                    nc.sync.dma_start(etmp[:], r_(e_ap[t * P : (t + 1) * P, :]))
                    eps = pps.tile([D, P], f32r, tag="eps")
                    nc.tensor.transpose(eps[:], etmp[:], ident[:])
                    nc.any.tensor_copy(et[:, t * P : (t + 1) * P], eps[:])

                # E rows for this core's nodes: ej[p, h*D+d] = E[h*128+p, d]
                ej = pp.tile([P, RT * D], f32, tag="ej")
                for h in range(RT):
                    nc.sync.dma_start(
                        ej[:, h * D : (h + 1) * D], e_ap[h * P : (h + 1) * P, :]
                    )

                bp_sb = pp.tile([D, COUT], f32r, tag="bp")
                nc.sync.dma_start(bp_sb[:], r_(bp_ap[:]))

                bias_sb = pp.tile([P, RT, COUT], f32, tag="bias")
                for h in range(RT):
                    bps = pps.tile([P, COUT], f32, tag="bps")
                    nc.tensor.matmul(
                        bps[:], et[:, h * P : (h + 1) * P], bp_sb[:],
                        start=True, stop=True,
                    )
                    nc.any.tensor_copy(bias_sb[:, h, :], bps[:])

                # weights [(k i), d, o] in bf16, with -I folded into k=0
                wp01 = pp.tile([P, D, COUT], bf16, tag="wp01")
                wp2 = pp.tile([CIN, D, COUT], bf16, tag="wp2")
                wk0 = pp.tile([CIN, D, COUT], f32, tag="wk0")
                wk2 = pp.tile([CIN, D, COUT], f32, tag="wk2")
                nc.sync.dma_start(
                    wk0[:], wp_ap[:, 0, :, :].rearrange("d i o -> i d o")
                )
                nc.sync.dma_start(
                    wk2[:], wp_ap[:, 2, :, :].rearrange("d i o -> i d o")
                )
                nc.vector.tensor_tensor(wp01[0:CIN, :, :], wk0[:], wk2[:], Alu.subtract)
                nc.any.tensor_copy(wp2[:], wk2[:])
                nc.sync.dma_start(
                    wk0[:], wp_ap[:, 1, :, :].rearrange("d i o -> i d o")
                )
                nc.any.tensor_copy(wp01[CIN:P, :, :], wk0[:])

            with tc.tile_pool(name="mid", bufs=1) as pm:
                # AT_col[m_part, s_tile, r] = A[R[r], m]; y2r = 2*(A^2)[R[r],m]
                at = pm.tile([P, NT, NL], f32r, tag="at")
                y2r = pm.tile([P, NT, NL], f32r, tag="y2r")

                with (
                    tc.tile_pool(name="apool", bufs=1) as pa,
                    tc.tile_pool(name="mpsum", bufs=2, space="PSUM") as mps,
                ):
                    a_sb = pa.tile([P, NT, N], f32r, tag="a")
                    dinv = pp.tile([P, NT], f32, tag="dinv")
                    dsum = pp.tile([P, NT], f32, tag="dsum")

                    # ---- A = rowsoftmax(relu(E E^T)) via M = exp(relu(G)) ----
                    for t in range(NT):
                        for cc in range(4):
                            gps = mps.tile([P, 512], f32, tag="gps")
                            nc.tensor.matmul(
                                gps[:],
                                et[:, t * P : (t + 1) * P],
                                et[:, cc * 512 : (cc + 1) * 512],
                                start=True, stop=True,
                            )
                            nc.vector.tensor_scalar_max(
                                a_sb[:, t, cc * 512 : (cc + 1) * 512], gps[:], 0.0
                            )
                        nc.scalar.activation(a_sb[:, t, :], a_sb[:, t, :], AFT.Exp)
                        nc.vector.reduce_sum(
                            dsum[:, t : t + 1], a_sb[:, t, :], axis=AX.X
                        )
                        nc.vector.reciprocal(dinv[:, t : t + 1], dsum[:, t : t + 1])
                        nc.vector.tensor_scalar_mul(
                            a_sb[:, t, :], a_sb[:, t, :], dinv[:, t : t + 1]
                        )

                    # ---- AT_col: transpose A's first RT row tiles ----
                    for h in range(RT):
                        for s in range(NT):
                            tps = mps.tile([P, P], f32r, tag="tps")
                            nc.tensor.transpose(
                                tps[:], a_sb[:, h, s * P : (s + 1) * P], ident[:]
                            )
                            nc.any.tensor_copy(at[:, s, h * P : (h + 1) * P], tps[:])

                    # ---- PT2[m, r] = (A^2)[R[r], m], scaled by 2 ----
                    for mt in range(NT):
                        pps2 = mps.tile([P, NL], f32, tag="pt2")
                        for s in range(NT):
                            nc.tensor.matmul(
                                pps2[:],
                                a_sb[:, s, mt * P : (mt + 1) * P],
                                at[:, s, :],
                                start=(s == 0), stop=(s == NT - 1),
                            )
                        nc.vector.tensor_scalar_mul(y2r[:, mt, :], pps2[:], 2.0)

                # ---- hops + Z ----
                with tc.tile_pool(name="hpool", bufs=1) as ph:
                    h01 = ph.tile([P, B, NL], bf16, tag="h01")
                    h2 = ph.tile([CIN, B, NL], bf16, tag="h2")

                    with (
                        tc.tile_pool(name="xpool", bufs=8) as px,
                        tc.tile_pool(name="hpsum", bufs=2, space="PSUM") as hps,
                    ):
                        for b in range(B):
                            ps1 = hps.tile([CIN, NL], f32, tag="ps1")
                            ps2 = hps.tile([CIN, NL], f32, tag="ps2")
                            for mc in range(NT):
                                xb = px.tile([P, CIN], f32r, tag="xb")
                                nc.sync.dma_start(
                                    xb[:], r_(x_ap[b, mc * P : (mc + 1) * P, :])
                                )
                                nc.tensor.matmul(
                                    ps1[:], xb[:], at[:, mc, :],
                                    start=(mc == 0), stop=(mc == NT - 1),
                                )
                                nc.tensor.matmul(
                                    ps2[:], xb[:], y2r[:, mc, :],
                                    start=(mc == 0), stop=(mc == NT - 1),
                                )
                                if mc < RT:
                                    xtp = hps.tile([CIN, P], f32r, tag="xtp")
                                    nc.tensor.transpose(xtp[:], xb[:], ident[:])
                                    nc.any.tensor_copy(
                                        h01[0:CIN, b, mc * P : (mc + 1) * P], xtp[:]
                                    )
                            nc.any.tensor_copy(h01[CIN:P, b, :], ps1[:])
                            nc.any.tensor_copy(h2[:, b, :], ps2[:])

                    # ---- Z + E-weighted d-sum + bias + out ----
                    with (
                        tc.tile_pool(name="zpool", bufs=2) as pz,
                        tc.tile_pool(name="accpool", bufs=3) as pacc,
                        tc.tile_pool(name="zpsum", bufs=4, space="PSUM") as zps,
                    ):
                        GQ = 4
                        for g in range(B // GQ * RT):
                            half = g % RT
                            b0 = (g // RT) * GQ
                            zw = pz.tile([P, GQ, DO], f32, tag="zw")
                            for q in range(GQ):
                                b = b0 + q
                                l01 = h01[:, b, half * P : (half + 1) * P]
                                l2 = h2[:, b, half * P : (half + 1) * P]
                                for nh in range(2):
                                    zp = zps.tile([P, 512], f32, tag="zp")
                                    nc.tensor.matmul(
                                        zp[:], l01,
                                        wp01[:, nh * 8 : (nh + 1) * 8, :],
                                        start=True, stop=False,
                                    )
                                    nc.tensor.matmul(
                                        zp[:], l2,
                                        wp2[:, nh * 8 : (nh + 1) * 8, :],
                                        start=False, stop=True,
                                    )
                                    nc.any.tensor_copy(
                                        zw[:, q, nh * 512 : (nh + 1) * 512], zp[:]
                                    )
                            acc = pacc.tile([P, GQ, COUT], f32, tag="acc")
                            nc.vector.tensor_copy(
                                acc[:],
                                bias_sb[:, half : half + 1, :].to_broadcast(
                                    [P, GQ, COUT]
                                ),
                            )
                            for d in range(D):
                                nc.vector.scalar_tensor_tensor(
                                    acc[:],
                                    zw[:, :, d * COUT : (d + 1) * COUT],
                                    ej[:, half * D + d : half * D + d + 1],
                                    acc[:],
                                    Alu.mult,
                                    Alu.add,
                                )
                            for q in range(GQ):
                                nc.sync.dma_start(
                                    out_ap[b0 + q, half * P : (half + 1) * P, :],
                                    acc[:, q, :],
                                )
    nc.compile()
    return nc


def kernel(X, E, weights_pool, bias_pool):
    global LAST_RESULT
    from concourse.bass_utils import run_bass_kernel_spmd

    if "nc" not in _CACHE:
        _CACHE["nc"] = _build_bass()
    nc = _CACHE["nc"]

    X = np.ascontiguousarray(X, dtype=np.float32)
    E = np.ascontiguousarray(E, dtype=np.float32)
    wp = np.ascontiguousarray(weights_pool, dtype=np.float32)
    bp = np.ascontiguousarray(bias_pool, dtype=np.float32)

    in_maps = []
    for j in range(N_CORES):
        in_maps.append(
            {
                "X": np.ascontiguousarray(np.roll(X, -NL * j, axis=1)),
                "E": np.ascontiguousarray(np.roll(E, -NL * j, axis=0)),
                "WP": wp,
                "BP": bp,
            }
        )

    res = run_bass_kernel_spmd(nc, in_maps, core_ids=list(range(N_CORES)))
    LAST_RESULT = res
    out = np.concatenate([res.results[j]["OUT"] for j in range(N_CORES)], axis=1)
    return out


if __name__ == "__main__":
    rng = np.random.default_rng(0)
    X = rng.standard_normal((B, N, CIN), dtype=np.float32)
    E = rng.standard_normal((N, D), dtype=np.float32)
    wp = rng.standard_normal((D, K, CIN, COUT), dtype=np.float32).astype(np.float32)
    bp = rng.standard_normal((D, COUT), dtype=np.float32)
    t0 = time.time()
    out = kernel(X, E, wp, bp)
    print("kernel done", out.shape, time.time() - t0)


# revision 11
# speedup vs baseline: 154.8689x; 154.8689x over previous
"""AVWGCN Bass kernel for 8 TRN2 NeuronCores.

Strategy: shard the node dimension N=2048 into 8 slices of 256. Host-side we
np.roll E (node rows) and X (node axis) per core so every core runs an
IDENTICAL program that computes output for its first 256 (rolled) nodes.

Math (per core, R = rolled nodes [0:256)):
  G = E@E.T (symmetric), M = exp(relu(G)) (symmetric), D = rowsum(M),
  A = M/D (row-softmax of relu(G) -- matches jax.nn.softmax up to max-shift).
  AT_col[m, r]   = A[R[r], m]           (PE transpose of A's first 2 row tiles)
  PT2[m, r]      = sum_s A[s,m]*AT_col[s,r] = (A^2)[R[r], m]   (no transposes!)
  Per batch b:   hop1[i, r] = sum_m X[b,m,i]*AT_col[m,r] = (A @ X_b)[R[r], i]
                 hop2[i, r] = sum_m X[b,m,i]*2*PT2[m,r] = 2(A^2 @ X_b)[R[r], i]
                 xt[i, r]   = X[b, R[r], i]             (PE transpose)
  H layout: H01[(k i)=128, b, r] = [xt | hop1], H2[(i)=64, b, r] = hop2
  Weights: Wp_eff[d,0] = Wp[d,0]-Wp[d,2] folds the "-I" of T2 = 2A^2 - I.
  Z: out_wide[r, (d o)] = sum_{ki} H[(ki),(b r)] * Wp_wide[(ki),(d o)]
  out[b, n=R[r], o] = bias[n,o] + sum_d E[n,d] * out_wide[r, (d,o)]
"""

import os
import sys
import time

sys.path.insert(0, "/opt/trn_rl_repo")

import numpy as np

N_CORES = 8
B, N, CIN, COUT, K, D = 64, 2048, 64, 64, 3, 16
NL = N // N_CORES  # 256 nodes per core
P = 128

_CACHE = {}
LAST_RESULT = None


def _build_bass():
    import concourse.bass as bass
    import concourse.mybir as mybir
    import concourse.tile as tile
    from concourse import bacc
    from concourse.masks import make_identity

    f32 = mybir.dt.float32
    f32r = mybir.dt.float32r
    Alu = mybir.AluOpType
    AFT = mybir.ActivationFunctionType
    AX = mybir.AxisListType

    def r_(ap):
        return ap.bitcast(f32r)

    nc = bacc.Bacc(
        "TRN2",
        target_bir_lowering=False,
        debug=False,
        enable_asserts=False,
        num_devices=N_CORES,
    )

    x_ap = nc.dram_tensor("X", [B, N, CIN], f32, kind="ExternalInput").ap()
    e_ap = nc.dram_tensor("E", [N, D], f32, kind="ExternalInput").ap()
    wp_ap = nc.dram_tensor("WP", [D, K, CIN, COUT], f32, kind="ExternalInput").ap()
    bp_ap = nc.dram_tensor("BP", [D, COUT], f32, kind="ExternalInput").ap()
    out_ap = nc.dram_tensor("OUT", [B, NL, COUT], f32, kind="ExternalOutput").ap()

    NT = N // P  # 16 row tiles
    DO = D * COUT  # 1024
    RT = NL // P  # 2 r-half tiles

    bf16 = mybir.dt.bfloat16

    with tile.TileContext(nc) as tc:
        with tc.tile_pool(name="persist", bufs=1) as pp:
            # ---- phase 0: constants / small tensors ----
            with tc.tile_pool(name="ppsum", bufs=2, space="PSUM") as pps:
                ident_f = pp.tile([P, P], f32, tag="ident_f")
                make_identity(nc, ident_f[:])
                ident = pp.tile([P, P], f32r, tag="ident")
                nc.vector.tensor_copy(ident[:], ident_f[:])

                et = pp.tile([D, N], f32r, tag="et")  # E.T [16, 2048]
                for t in range(NT):
                    etmp = pp.tile([P, D], f32r, tag="etmp")
                    nc.sync.dma_start(etmp[:], r_(e_ap[t * P : (t + 1) * P, :]))
                    eps = pps.tile([D, P], f32r, tag="eps")
                    nc.tensor.transpose(eps[:], etmp[:], ident[:])
                    nc.any.tensor_copy(et[:, t * P : (t + 1) * P], eps[:])

                # E rows for this core's nodes: ej[p, h*D+d] = E[h*128+p, d]
                ej = pp.tile([P, RT * D], f32, tag="ej")
                for h in range(RT):
                    nc.sync.dma_start(
                        ej[:, h * D : (h + 1) * D], e_ap[h * P : (h + 1) * P, :]
                    )

                bp_sb = pp.tile([D, COUT], f32r, tag="bp")
                nc.sync.dma_start(bp_sb[:], r_(bp_ap[:]))

                bias_sb = pp.tile([P, RT, COUT], f32, tag="bias")
                for h in range(RT):
                    bps = pps.tile([P, COUT], f32, tag="bps")
                    nc.tensor.matmul(
                        bps[:], et[:, h * P : (h + 1) * P], bp_sb[:],
                        start=True, stop=True,
                    )
                    nc.any.tensor_copy(bias_sb[:, h, :], bps[:])

                # weights [(k i), d, o] in bf16, with -I folded into k=0
                wp01 = pp.tile([P, D, COUT], bf16, tag="wp01")
                wp2 = pp.tile([CIN, D, COUT], bf16, tag="wp2")
                wk0 = pp.tile([CIN, D, COUT], f32, tag="wk0")
                wk2 = pp.tile([CIN, D, COUT], f32, tag="wk2")
                nc.sync.dma_start(
                    wk0[:], wp_ap[:, 0, :, :].rearrange("d i o -> i d o")
                )
                nc.sync.dma_start(
                    wk2[:], wp_ap[:, 2, :, :].rearrange("d i o -> i d o")
                )
                nc.vector.tensor_tensor(wp01[0:CIN, :, :], wk0[:], wk2[:], Alu.subtract)
                nc.any.tensor_copy(wp2[:], wk2[:])
                nc.sync.dma_start(
                    wk0[:], wp_ap[:, 1, :, :].rearrange("d i o -> i d o")
                )
                nc.any.tensor_copy(wp01[CIN:P, :, :], wk0[:])

            with tc.tile_pool(name="mid", bufs=1) as pm:
                # AT_col[m_part, s_tile, r] = A[R[r], m]; y2r = 2*(A^2)[R[r],m]
                at = pm.tile([P, NT, NL], f32r, tag="at")
                y2r = pm.tile([P, NT, NL], f32r, tag="y2r")

                with (
                    tc.tile_pool(name="apool", bufs=1) as pa,
                    tc.tile_pool(name="mpsum", bufs=2, space="PSUM") as mps,
                ):
                    a_sb = pa.tile([P, NT, N], f32r, tag="a")
                    dinv = pp.tile([P, NT], f32, tag="dinv")
                    dsum = pp.tile([P, NT], f32, tag="dsum")

                    # ---- A = rowsoftmax(relu(E E^T)) via M = exp(relu(G)) ----
                    for t in range(NT):
                        for cc in range(4):
                            gps = mps.tile([P, 512], f32, tag="gps")
                            nc.tensor.matmul(
                                gps[:],
                                et[:, t * P : (t + 1) * P],
                                et[:, cc * 512 : (cc + 1) * 512],
                                start=True, stop=True,
                            )
                            nc.vector.tensor_scalar_max(
                                a_sb[:, t, cc * 512 : (cc + 1) * 512], gps[:], 0.0
                            )
                        nc.scalar.activation(a_sb[:, t, :], a_sb[:, t, :], AFT.Exp)
                        nc.vector.reduce_sum(
                            dsum[:, t : t + 1], a_sb[:, t, :], axis=AX.X
                        )
                        nc.vector.reciprocal(dinv[:, t : t + 1], dsum[:, t : t + 1])
                        nc.vector.tensor_scalar_mul(
                            a_sb[:, t, :], a_sb[:, t, :], dinv[:, t : t + 1]
                        )

                    # ---- AT_col: transpose A's first RT row tiles ----
                    for h in range(RT):
                        for s in range(NT):
                            tps = mps.tile([P, P], f32r, tag="tps")
                            nc.tensor.transpose(
                                tps[:], a_sb[:, h, s * P : (s + 1) * P], ident[:]
                            )
                            nc.any.tensor_copy(at[:, s, h * P : (h + 1) * P], tps[:])

                    # ---- PT2[m, r] = (A^2)[R[r], m], scaled by 2 ----
                    for mt in range(NT):
                        pps2 = mps.tile([P, NL], f32, tag="pt2")
                        for s in range(NT):
                            nc.tensor.matmul(
                                pps2[:],
                                a_sb[:, s, mt * P : (mt + 1) * P],
                                at[:, s, :],
                                start=(s == 0), stop=(s == NT - 1),
                            )
                        nc.vector.tensor_scalar_mul(y2r[:, mt, :], pps2[:], 2.0)

                # ---- hops + Z ----
                with tc.tile_pool(name="hpool", bufs=1) as ph:
                    h01 = ph.tile([P, B, NL], bf16, tag="h01")
                    h2 = ph.tile([CIN, B, NL], bf16, tag="h2")

                    with (
                        tc.tile_pool(name="xpool", bufs=8) as px,
                        tc.tile_pool(name="hpsum", bufs=2, space="PSUM") as hps,
                    ):
                        # batch pairs packed along lhsT free dim: psum rows
                        # 0:64 = b0's channels, 64:128 = b1's channels
                        for bp2 in range(B // 2):
                            b0, b1 = 2 * bp2, 2 * bp2 + 1
                            ps1 = hps.tile([P, NL], f32, tag="ps1")
                            ps2 = hps.tile([P, NL], f32, tag="ps2")
                            for mc in range(NT):
                                xb = px.tile([P, 2 * CIN], f32r, tag="xb")
                                nc.sync.dma_start(
                                    xb[:, 0:CIN],
                                    r_(x_ap[b0, mc * P : (mc + 1) * P, :]),
                                )
                                nc.sync.dma_start(
                                    xb[:, CIN : 2 * CIN],
                                    r_(x_ap[b1, mc * P : (mc + 1) * P, :]),
                                )
                                st = dict(start=(mc == 0), stop=(mc == NT - 1))
                                nc.tensor.matmul(
                                    ps1[:], xb[:], at[:, mc, :], **st
                                )
                                nc.tensor.matmul(
                                    ps2[:], xb[:], y2r[:, mc, :], **st
                                )
                                if mc < RT:
                                    xtp = hps.tile([P, P], f32r, tag="xtp")
                                    nc.tensor.transpose(xtp[:], xb[:], ident[:])
                                    nc.any.tensor_copy(
                                        h01[0:CIN, b0, mc * P : (mc + 1) * P],
                                        xtp[0:CIN, :],
                                    )
                                    nc.any.tensor_copy(
                                        h01[0:CIN, b1, mc * P : (mc + 1) * P],
                                        xtp[CIN:P, :],
                                    )
                            nc.any.tensor_copy(h01[CIN:P, b0, :], ps1[0:CIN, :])
                            nc.any.tensor_copy(h01[CIN:P, b1, :], ps1[CIN:P, :])
                            nc.any.tensor_copy(h2[:, b0, :], ps2[0:CIN, :])
                            nc.any.tensor_copy(h2[:, b1, :], ps2[CIN:P, :])

                    # ---- Z + E-weighted d-sum + bias + out ----
                    with (
                        tc.tile_pool(name="zpool", bufs=2) as pz,
                        tc.tile_pool(name="accpool", bufs=3) as pacc,
                        tc.tile_pool(name="zpsum", bufs=4, space="PSUM") as zps,
                    ):
                        GQ = 4
                        for g in range(B // GQ * RT):
                            half = g % RT
                            b0 = (g // RT) * GQ
                            zw = pz.tile([P, GQ, DO], f32, tag="zw")
                            for q in range(GQ):
                                b = b0 + q
                                l01 = h01[:, b, half * P : (half + 1) * P]
                                l2 = h2[:, b, half * P : (half + 1) * P]
                                for nh in range(2):
                                    zp = zps.tile([P, 512], f32, tag="zp")
                                    nc.tensor.matmul(
                                        zp[:], l01,
                                        wp01[:, nh * 8 : (nh + 1) * 8, :],
                                        start=True, stop=False,
                                    )
                                    nc.tensor.matmul(
                                        zp[:], l2,
                                        wp2[:, nh * 8 : (nh + 1) * 8, :],
                                        start=False, stop=True,
                                    )
                                    nc.any.tensor_copy(
                                        zw[:, q, nh * 512 : (nh + 1) * 512], zp[:]
                                    )
                            acc = pacc.tile([P, GQ, COUT], f32, tag="acc")
                            nc.vector.tensor_copy(
                                acc[:],
                                bias_sb[:, half : half + 1, :].to_broadcast(
                                    [P, GQ, COUT]
                                ),
                            )
                            for d in range(D):
                                nc.vector.scalar_tensor_tensor(
                                    acc[:],
                                    zw[:, :, d * COUT : (d + 1) * COUT],
                                    ej[:, half * D + d : half * D + d + 1],
                                    acc[:],
                                    Alu.mult,
                                    Alu.add,
                                )
                            for q in range(GQ):
                                nc.sync.dma_start(
                                    out_ap[b0 + q, half * P : (half + 1) * P, :],
                                    acc[:, q, :],
                                )
    nc.compile()
    return nc


def kernel(X, E, weights_pool, bias_pool):
    global LAST_RESULT
    from concourse.bass_utils import run_bass_kernel_spmd

    if "nc" not in _CACHE:
        _CACHE["nc"] = _build_bass()
    nc = _CACHE["nc"]

    X = np.ascontiguousarray(X, dtype=np.float32)
    E = np.ascontiguousarray(E, dtype=np.float32)
    wp = np.ascontiguousarray(weights_pool, dtype=np.float32)
    bp = np.ascontiguousarray(bias_pool, dtype=np.float32)

    in_maps = []
    for j in range(N_CORES):
        in_maps.append(
            {
                "X": np.ascontiguousarray(np.roll(X, -NL * j, axis=1)),
                "E": np.ascontiguousarray(np.roll(E, -NL * j, axis=0)),
                "WP": wp,
                "BP": bp,
            }
        )

    res = run_bass_kernel_spmd(nc, in_maps, core_ids=list(range(N_CORES)))
    LAST_RESULT = res
    out = np.concatenate([res.results[j]["OUT"] for j in range(N_CORES)], axis=1)
    return out


if __name__ == "__main__":
    rng = np.random.default_rng(0)
    X = rng.standard_normal((B, N, CIN), dtype=np.float32)
    E = rng.standard_normal((N, D), dtype=np.float32)
    wp = rng.standard_normal((D, K, CIN, COUT), dtype=np.float32).astype(np.float32)
    bp = rng.standard_normal((D, COUT), dtype=np.float32)
    t0 = time.time()
    out = kernel(X, E, wp, bp)
    print("kernel done", out.shape, time.time() - t0)


# revision 14
# speedup vs baseline: 155.6346x; 1.0049x over previous
"""AVWGCN Bass kernel for 8 TRN2 NeuronCores.

Strategy: shard the node dimension N=2048 into 8 slices of 256. Host-side we
np.roll E (node rows) and X (node axis) per core so every core runs an
IDENTICAL program that computes output for its first 256 (rolled) nodes.

Math (per core, R = rolled nodes [0:256)):
  G = E@E.T (symmetric), M = exp(relu(G)) (symmetric), D = rowsum(M),
  A = M/D (row-softmax of relu(G) -- matches jax.nn.softmax up to max-shift).
  AT_col[m, r]   = A[R[r], m]           (PE transpose of A's first 2 row tiles)
  PT2[m, r]      = sum_s A[s,m]*AT_col[s,r] = (A^2)[R[r], m]   (no transposes!)
  Per batch b:   hop1[i, r] = sum_m X[b,m,i]*AT_col[m,r] = (A @ X_b)[R[r], i]
                 hop2[i, r] = sum_m X[b,m,i]*2*PT2[m,r] = 2(A^2 @ X_b)[R[r], i]
                 xt[i, r]   = X[b, R[r], i]             (PE transpose)
  H layout: H01[(k i)=128, b, r] = [xt | hop1], H2[(i)=64, b, r] = hop2
  Weights: Wp_eff[d,0] = Wp[d,0]-Wp[d,2] folds the "-I" of T2 = 2A^2 - I.
  Z: out_wide[r, (d o)] = sum_{ki} H[(ki),(b r)] * Wp_wide[(ki),(d o)]
  out[b, n=R[r], o] = bias[n,o] + sum_d E[n,d] * out_wide[r, (d,o)]
"""

import os
import sys
import time

sys.path.insert(0, "/opt/trn_rl_repo")

import numpy as np

N_CORES = 8
B, N, CIN, COUT, K, D = 64, 2048, 64, 64, 3, 16
NL = N // N_CORES  # 256 nodes per core
P = 128

_CACHE = {}
LAST_RESULT = None


def _build_bass():
    import concourse.bass as bass
    import concourse.mybir as mybir
    import concourse.tile as tile
    from concourse import bacc
    from concourse.masks import make_identity

    f32 = mybir.dt.float32
    f32r = mybir.dt.float32r
    Alu = mybir.AluOpType
    AFT = mybir.ActivationFunctionType
    AX = mybir.AxisListType

    def r_(ap):
        return ap.bitcast(f32r)

    nc = bacc.Bacc(
        "TRN2",
        target_bir_lowering=False,
        debug=False,
        enable_asserts=False,
        num_devices=N_CORES,
    )

    x_ap = nc.dram_tensor("X", [B, N, CIN], f32, kind="ExternalInput").ap()
    e_ap = nc.dram_tensor("E", [N, D], f32, kind="ExternalInput").ap()
    wp_ap = nc.dram_tensor("WP", [D, K, CIN, COUT], f32, kind="ExternalInput").ap()
    bp_ap = nc.dram_tensor("BP", [D, COUT], f32, kind="ExternalInput").ap()
    out_ap = nc.dram_tensor("OUT", [B, NL, COUT], f32, kind="ExternalOutput").ap()

    NT = N // P  # 16 row tiles
    DO = D * COUT  # 1024
    RT = NL // P  # 2 r-half tiles

    bf16 = mybir.dt.bfloat16

    with tile.TileContext(nc) as tc:
        with tc.tile_pool(name="persist", bufs=1) as pp:
            # ---- phase 0: constants / small tensors ----
            with tc.tile_pool(name="ppsum", bufs=2, space="PSUM") as pps:
                ident_f = pp.tile([P, P], f32, tag="ident_f")
                make_identity(nc, ident_f[:])
                ident = pp.tile([P, P], f32r, tag="ident")
                nc.vector.tensor_copy(ident[:], ident_f[:])

                et = pp.tile([D, N], f32r, tag="et")  # E.T [16, 2048]
                for t in range(NT):
                    etmp = pp.tile([P, D], f32r, tag="etmp")
                    nc.sync.dma_start(etmp[:], r_(e_ap[t * P : (t + 1) * P, :]))
                    eps = pps.tile([D, P], f32r, tag="eps")
                    nc.tensor.transpose(eps[:], etmp[:], ident[:])
                    nc.any.tensor_copy(et[:, t * P : (t + 1) * P], eps[:])

                # E rows for this core's nodes: ej[p, h*D+d] = E[h*128+p, d]
                ej = pp.tile([P, RT * D], f32, tag="ej")
                for h in range(RT):
                    nc.sync.dma_start(
                        ej[:, h * D : (h + 1) * D], e_ap[h * P : (h + 1) * P, :]
                    )

                bp_sb = pp.tile([D, COUT], f32r, tag="bp")
                nc.sync.dma_start(bp_sb[:], r_(bp_ap[:]))

                bias_sb = pp.tile([P, RT, COUT], f32, tag="bias")
                for h in range(RT):
                    bps = pps.tile([P, COUT], f32, tag="bps")
                    nc.tensor.matmul(
                        bps[:], et[:, h * P : (h + 1) * P], bp_sb[:],
                        start=True, stop=True,
                    )
                    nc.any.tensor_copy(bias_sb[:, h, :], bps[:])

                # weights [(k i), d, o] in bf16, with -I folded into k=0
                wp01 = pp.tile([P, D, COUT], bf16, tag="wp01")
                wp2 = pp.tile([CIN, D, COUT], bf16, tag="wp2")
                wk0 = pp.tile([CIN, D, COUT], f32, tag="wk0")
                wk2 = pp.tile([CIN, D, COUT], f32, tag="wk2")
                nc.sync.dma_start(
                    wk0[:], wp_ap[:, 0, :, :].rearrange("d i o -> i d o")
                )
                nc.sync.dma_start(
                    wk2[:], wp_ap[:, 2, :, :].rearrange("d i o -> i d o")
                )
                nc.vector.tensor_tensor(wp01[0:CIN, :, :], wk0[:], wk2[:], Alu.subtract)
                nc.any.tensor_copy(wp2[:], wk2[:])
                nc.sync.dma_start(
                    wk0[:], wp_ap[:, 1, :, :].rearrange("d i o -> i d o")
                )
                nc.any.tensor_copy(wp01[CIN:P, :, :], wk0[:])

            with tc.tile_pool(name="mid", bufs=1) as pm:
                # AT_col[m_part, s_tile, r] = A[R[r], m]; y2r = 2*(A^2)[R[r],m]
                at = pm.tile([P, NT, NL], f32r, tag="at")
                y2r = pm.tile([P, NT, NL], f32r, tag="y2r")

                with (
                    tc.tile_pool(name="apool", bufs=1) as pa,
                    tc.tile_pool(name="mpsum", bufs=2, space="PSUM") as mps,
                ):
                    a_sb = pa.tile([P, NT, N], f32r, tag="a")
                    dinv = pp.tile([P, NT], f32, tag="dinv")
                    dsum = pp.tile([P, NT], f32, tag="dsum")

                    # ---- A = rowsoftmax(relu(E E^T)) via M = exp(relu(G)) ----
                    for t in range(NT):
                        for cc in range(4):
                            gps = mps.tile([P, 512], f32, tag="gps")
                            nc.tensor.matmul(
                                gps[:],
                                et[:, t * P : (t + 1) * P],
                                et[:, cc * 512 : (cc + 1) * 512],
                                start=True, stop=True,
                            )
                            nc.vector.tensor_scalar_max(
                                a_sb[:, t, cc * 512 : (cc + 1) * 512], gps[:], 0.0
                            )
                        nc.scalar.activation(a_sb[:, t, :], a_sb[:, t, :], AFT.Exp)
                        nc.vector.reduce_sum(
                            dsum[:, t : t + 1], a_sb[:, t, :], axis=AX.X
                        )
                        nc.vector.reciprocal(dinv[:, t : t + 1], dsum[:, t : t + 1])
                        nc.vector.tensor_scalar_mul(
                            a_sb[:, t, :], a_sb[:, t, :], dinv[:, t : t + 1]
                        )

                    # ---- AT_col: transpose A's first RT row tiles ----
                    for h in range(RT):
                        for s in range(NT):
                            tps = mps.tile([P, P], f32r, tag="tps")
                            nc.tensor.transpose(
                                tps[:], a_sb[:, h, s * P : (s + 1) * P], ident[:]
                            )
                            nc.any.tensor_copy(at[:, s, h * P : (h + 1) * P], tps[:])

                    # ---- PT2[m, r] = (A^2)[R[r], m], scaled by 2 ----
                    for mt in range(NT):
                        pps2 = mps.tile([P, NL], f32, tag="pt2")
                        for s in range(NT):
                            nc.tensor.matmul(
                                pps2[:],
                                a_sb[:, s, mt * P : (mt + 1) * P],
                                at[:, s, :],
                                start=(s == 0), stop=(s == NT - 1),
                            )
                        nc.vector.tensor_scalar_mul(y2r[:, mt, :], pps2[:], 2.0)

                # ---- hops + Z ----
                with tc.tile_pool(name="hpool", bufs=1) as ph:
                    # per-batch tiles so Z can start while later hops run
                    h01 = [
                        ph.tile([P, NL], bf16, tag=f"h01_{b}", name=f"h01_{b}")
                        for b in range(B)
                    ]
                    h2 = [
                        ph.tile([CIN, NL], bf16, tag=f"h2_{b}", name=f"h2_{b}")
                        for b in range(B)
                    ]

                    with (
                        tc.tile_pool(name="xpool", bufs=3) as px,
                        tc.tile_pool(name="hpsum", bufs=2, space="PSUM") as hps,
                    ):
                        # batch pairs packed along lhsT free dim: psum rows
                        # 0:64 = b0's channels, 64:128 = b1's channels
                        for bp2 in range(B // 2):
                            b0, b1 = 2 * bp2, 2 * bp2 + 1
                            ps1 = hps.tile([P, NL], f32, tag="ps1")
                            ps2 = hps.tile([P, NL], f32, tag="ps2")
                            xpair = px.tile([P, NT, 2, CIN], f32r, tag="xb")
                            nc.sync.dma_start(
                                xpair[:, :, 0, :],
                                r_(x_ap[b0].rearrange("(mc p) c -> p mc c", p=P)),
                            )
                            nc.sync.dma_start(
                                xpair[:, :, 1, :],
                                r_(x_ap[b1].rearrange("(mc p) c -> p mc c", p=P)),
                            )
                            for mc in range(NT):
                                xb = xpair[:, mc, :, :]
                                st = dict(start=(mc == 0), stop=(mc == NT - 1))
                                nc.tensor.matmul(
                                    ps1[:], xb, at[:, mc, :], **st
                                )
                                nc.tensor.matmul(
                                    ps2[:], xb, y2r[:, mc, :], **st
                                )
                                if mc < RT:
                                    xtp = hps.tile([P, P], f32r, tag="xtp")
                                    nc.tensor.transpose(xtp[:], xb, ident[:])
                                    nc.any.tensor_copy(
                                        h01[b0][0:CIN, mc * P : (mc + 1) * P],
                                        xtp[0:CIN, :],
                                    )
                                    nc.any.tensor_copy(
                                        h01[b1][0:CIN, mc * P : (mc + 1) * P],
                                        xtp[CIN:P, :],
                                    )
                            nc.any.tensor_copy(h01[b0][CIN:P, :], ps1[0:CIN, :])
                            nc.any.tensor_copy(h01[b1][CIN:P, :], ps1[CIN:P, :])
                            nc.any.tensor_copy(h2[b0][:], ps2[0:CIN, :])
                            nc.any.tensor_copy(h2[b1][:], ps2[CIN:P, :])

                    # ---- Z + E-weighted d-sum + bias + out ----
                    with (
                        tc.tile_pool(name="zpool", bufs=2) as pz,
                        tc.tile_pool(name="accpool", bufs=3) as pacc,
                        tc.tile_pool(name="zpsum", bufs=4, space="PSUM") as zps,
                    ):
                        GQ = 4
                        for g in range(B // GQ * RT):
                            half = g % RT
                            b0 = (g // RT) * GQ
                            zw = pz.tile([P, GQ, DO], f32, tag="zw")
                            for q in range(GQ):
                                b = b0 + q
                                l01 = h01[b][:, half * P : (half + 1) * P]
                                l2 = h2[b][:, half * P : (half + 1) * P]
                                for nh in range(2):
                                    zp = zps.tile([P, 512], f32, tag="zp")
                                    nc.tensor.matmul(
                                        zp[:], l01,
                                        wp01[:, nh * 8 : (nh + 1) * 8, :],
                                        start=True, stop=False,
                                    )
                                    nc.tensor.matmul(
                                        zp[:], l2,
                                        wp2[:, nh * 8 : (nh + 1) * 8, :],
                                        start=False, stop=True,
                                    )
                                    nc.any.tensor_copy(
                                        zw[:, q, nh * 512 : (nh + 1) * 512], zp[:]
                                    )
                            acc = pacc.tile([P, GQ, COUT], f32, tag="acc")
                            nc.vector.tensor_copy(
                                acc[:],
                                bias_sb[:, half : half + 1, :].to_broadcast(
                                    [P, GQ, COUT]
                                ),
                            )
                            for d in range(D):
                                nc.vector.scalar_tensor_tensor(
                                    acc[:],
                                    zw[:, :, d * COUT : (d + 1) * COUT],
                                    ej[:, half * D + d : half * D + d + 1],
                                    acc[:],
                                    Alu.mult,
                                    Alu.add,
                                )
                            nc.sync.dma_start(
                                out_ap[
                                    b0 : b0 + GQ, half * P : (half + 1) * P, :
                                ].rearrange("q p o -> p q o"),
                                acc[:],
                            )
    nc.compile()
    return nc


def kernel(X, E, weights_pool, bias_pool):
    global LAST_RESULT
    from concourse.bass_utils import run_bass_kernel_spmd

    if "nc" not in _CACHE:
        _CACHE["nc"] = _build_bass()
    nc = _CACHE["nc"]

    X = np.ascontiguousarray(X, dtype=np.float32)
    E = np.ascontiguousarray(E, dtype=np.float32)
    wp = np.ascontiguousarray(weights_pool, dtype=np.float32)
    bp = np.ascontiguousarray(bias_pool, dtype=np.float32)

    in_maps = []
    for j in range(N_CORES):
        in_maps.append(
            {
                "X": np.ascontiguousarray(np.roll(X, -NL * j, axis=1)),
                "E": np.ascontiguousarray(np.roll(E, -NL * j, axis=0)),
                "WP": wp,
                "BP": bp,
            }
        )

    res = run_bass_kernel_spmd(nc, in_maps, core_ids=list(range(N_CORES)))
    LAST_RESULT = res
    out = np.concatenate([res.results[j]["OUT"] for j in range(N_CORES)], axis=1)
    return out


if __name__ == "__main__":
    rng = np.random.default_rng(0)
    X = rng.standard_normal((B, N, CIN), dtype=np.float32)
    E = rng.standard_normal((N, D), dtype=np.float32)
    wp = rng.standard_normal((D, K, CIN, COUT), dtype=np.float32).astype(np.float32)
    bp = rng.standard_normal((D, COUT), dtype=np.float32)
    t0 = time.time()
    out = kernel(X, E, wp, bp)
    print("kernel done", out.shape, time.time() - t0)
